# revision 9
# baseline (speedup 1.0000x reference)
"""Multi-head causal attention on 8 TRN2 NeuronCores.

Sharding: core c -> (batch b = c//2, head-group g = c%2). Each core computes
Q/K/V projections for its 8 heads (512 of the 1024 channels), causal
attention, and the row-parallel W_o partial product; the host sums the two
partials per batch (the "all-reduce").

Device layouts (per core):
  xT   (1024, 2048) bf16   x[b] transposed (channels on partitions)
  wqT  (1024, 512)  bf16   W_q[rows g].T  -> lhsT for QT = Wq_g @ xT
  wkT  (1024, 512)  bf16   same for K
  wvT  (1024, 512)  bf16   rhs for natural-layout V = x @ Wv_g.T
  woT  (512, 1024)  bf16   W_o[:, cols g].T -> lhsT for yT = Wo_g @ O^T
  mask (128, 256)   bf16   the 128x128 causal triangle (q>=k), 2 head copies
  yT   (1024, 2048) bf16   partial output, transposed

Attention per head h (d_k=64): scores are computed transposed,
S^T = K_h @ Q_h^T (k on partitions, q on free axis), exp on the scalar
engine (no max subtraction: |scores/8| < ~6 at these scales), multiplicative
0/1 mask on the single ragged 128-col sub-block of diagonal tiles, and P^T
is consumed directly as the moving operand of out^T = [V_h | 1]^T @ P^T,
whose row 64 accumulates the softmax denominators Z. Diagonal-crossing
blocks are computed only on their valid column range. Heads run in pairs
(partition offsets 0/64) so the two K=64 score matmuls occupy disjoint PE
row-groups concurrently.

v2 pipeline: head-pair 0's attention blocks are interleaved into the input
chunk loop, so the scalar engine starts its ~190us of exp work ~15us into
the kernel instead of ~80us. The attention inner loop is depth-2 software
pipelined (S(k0+1) is emitted before AV(k0)) and pulls "filler" matmuls
(later pairs' Q^T/K^T projections, then W_o tiles) to keep the PE busy
through the S -> exp -> mask -> AV latency chain; a starved PE also gets
re-throttled to 1.2 GHz by the HAM clock gate, so density matters twice.
"""

from collections import deque

import numpy as np

B, T, D = 4, 2048, 1024
NH, DK = 16, 64
NCORES = 8
HPC = NH // 2            # heads per core
HD = HPC * DK            # 512 head-dim channels per core
P = 128                  # partitions
NT = T // P              # 16 k-tiles
NQ = T // 512            # 4 q-blocks

_CACHE = {}


def _build():
    import concourse.mybir as mybir
    import concourse.tile as tile
    from concourse import bacc
    from concourse.tile import add_dep_helper

    f32, bf16 = mybir.dt.float32, mybir.dt.bfloat16
    Exp = mybir.ActivationFunctionType.Exp

    nc = bacc.Bacc(None, target_bir_lowering=False, debug=False)
    xT = nc.dram_tensor("xT", [D, T], bf16, kind="ExternalInput")
    wqT = nc.dram_tensor("wqT", [D, HD], bf16, kind="ExternalInput")
    wkT = nc.dram_tensor("wkT", [D, HD], bf16, kind="ExternalInput")
    wvT = nc.dram_tensor("wvT", [D, HD], bf16, kind="ExternalInput")
    woT = nc.dram_tensor("woT", [HD, D], bf16, kind="ExternalInput")
    mask = nc.dram_tensor("mask", [P, 2 * P], bf16, kind="ExternalInput")
    yT = nc.dram_tensor("yT", [D, T], bf16, kind="ExternalOutput")

    with tile.TileContext(nc) as tc:
        with (
            tc.tile_pool(name="persist", bufs=1) as persist,
            tc.tile_pool(name="work", bufs=6) as work,
            tc.tile_pool(name="psum", bufs=4, space="PSUM") as psum,
            tc.tile_pool(name="psum2", bufs=2, space="PSUM") as psum2,
        ):
            # ---- persistent tiles --------------------------------------
            xtc = [
                [persist.tile([P, 512], bf16, tag=f"x{c}_{t}", name=f"x{c}_{t}")
                 for t in range(NQ)]
                for c in range(8)
            ]
            wq_sb = persist.tile([P, 8, HD], bf16, tag="wq")
            wk_sb = persist.tile([P, 8, HD], bf16, tag="wk")
            wv_sb = persist.tile([P, 8, HD], bf16, tag="wv")
            wo_sb = persist.tile([P, 4, D], bf16, tag="wo")
            mask_sb = persist.tile([P, 2, P], bf16, tag="mask")
            qt = [persist.tile([P, T], bf16, tag=f"qt{a}", name=f"qt{a}")
                  for a in range(4)]
            kt = [persist.tile([P, T], bf16, tag=f"kt{a}", name=f"kt{a}")
                  for a in range(4)]
            vt = [persist.tile([P, HPC, DK + 1], bf16, tag=f"v{tt}", name=f"v{tt}")
                  for tt in range(NT)]
            otn = [persist.tile([P, T], bf16, tag=f"otn{i}", name=f"otn{i}")
                   for i in range(4)]

            # ---- input DMAs -------------------------------------------
            # xT streams in t-chunk-major order (per-c chains) so the
            # upfront V / Q^T / K^T matmuls can start on early chunks
            # instead of waiting for the whole 4MB transfer. wq/wk/mask
            # follow chunk 0 (needed by the first attention block); wo
            # follows chunk 1 (not needed until the a=3 phase).
            nc.sync.dma_start(out=wv_sb, in_=wvT.rearrange("(co p) d -> p co d", p=P))
            xT_r = xT.rearrange("(co p) t -> co p t", p=P)
            xdma = {}

            def emit_x_chunk(tch):
                for c in range(8):
                    ins = nc.sync.dma_start(
                        out=xtc[c][tch],
                        in_=xT_r[c][:, 512 * tch:512 * tch + 512],
                    )
                    if tch > 0:
                        add_dep_helper(ins.ins, xdma[c, tch - 1], sync=True,
                                       reason="x chunks in order")
                    xdma[c, tch] = ins.ins

            def emit_w(dst, srcp, gate):
                if srcp is None:
                    ins = nc.sync.dma_start(
                        out=mask_sb,
                        in_=mask.rearrange("p (g q) -> p g q", g=2),
                    )
                else:
                    ins = nc.sync.dma_start(
                        out=dst, in_=srcp.rearrange("(co p) d -> p co d", p=P)
                    )
                add_dep_helper(ins.ins, xdma[gate], sync=True,
                               reason="weights after early xT chunks")

            emit_x_chunk(0)
            emit_w(wq_sb, wqT, (7, 0))
            emit_w(wk_sb, wkT, (7, 0))
            emit_w(mask_sb, None, (7, 0))
            emit_x_chunk(1)
            emit_w(wo_sb, woT, (7, 1))
            emit_x_chunk(2)
            emit_x_chunk(3)
            for tt in range(NT):
                nc.vector.memset(vt[tt][:, :, DK:DK + 1], 1.0)

            # ---- PE warmup: dummy matmuls on scratch data while the
            # input DMA streams, so the HAM clock gate un-throttles the
            # PE (1.2 -> 2.4 GHz takes ~3.4us of sustained activity)
            # before the first real matmul arrives.
            warm = persist.tile([P, 512], bf16, tag="warm")
            nc.vector.memset(warm, 0.0)
            for i in range(28):
                wps = psum.tile([P, 512], f32, tag="ps", name=f"warm{i}")
                nc.tensor.matmul(wps, lhsT=warm[:, 0:128], rhs=warm,
                                 start=True, stop=True)

            # ---- op builders (each closure emits one PE matmul) --------
            def v_tile_ops(tt):
                st = {}

                def mk(c):
                    def op():
                        if c == 0:
                            st["ps"] = psum.tile([P, HD], f32, tag="ps",
                                                 name=f"vps{tt}")
                        nc.tensor.matmul(
                            st["ps"],
                            lhsT=xtc[c][tt // 4][:, P * (tt % 4):P * (tt % 4) + P],
                            rhs=wv_sb[:, c, :],
                            start=(c == 0),
                            stop=(c == 7),
                        )
                        if c == 7:
                            nc.vector.tensor_copy(
                                vt[tt][:, :, 0:DK],
                                st["ps"].rearrange("p (h e) -> p h e", e=DK),
                            )
                    return op

                return [mk(c) for c in range(8)]

            def proj_tile_ops(nm, w_sb, out_sb, a, tch):
                st = {}

                def mk(c):
                    def op():
                        if c == 0:
                            st["ps"] = psum.tile([P, 512], f32, tag="ps",
                                                 name=f"{nm}ps{a}_{tch}")
                        nc.tensor.matmul(
                            st["ps"],
                            lhsT=w_sb[:, c, 128 * a:128 * a + 128],
                            rhs=xtc[c][tch],
                            start=(c == 0),
                            stop=(c == 7),
                        )
                        if c == 7:
                            nc.vector.tensor_copy(
                                out_sb[:, 512 * tch:512 * tch + 512], st["ps"]
                            )
                    return op

                return [mk(c) for c in range(8)]

            def wo_tile_ops(dt_, tch):
                st = {}

                def mk(c):
                    def op():
                        if c == 0:
                            st["ps"] = psum.tile([P, 512], f32, tag="ps",
                                                 name=f"yps{dt_}_{tch}")
                        nc.tensor.matmul(
                            st["ps"],
                            lhsT=wo_sb[:, c, 128 * dt_:128 * dt_ + 128],
                            rhs=otn[c][:, 512 * tch:512 * tch + 512],
                            start=(c == 0),
                            stop=(c == 3),
                        )
                        if c == 3:
                            yst = work.tile([P, 512], bf16, tag="yst", bufs=3,
                                            name=f"yst{dt_}_{tch}")
                            nc.vector.tensor_copy(yst, st["ps"])
                            nc.sync.dma_start(
                                out=yT[128 * dt_:128 * dt_ + 128,
                                       512 * tch:512 * tch + 512],
                                in_=yst,
                            )
                    return op

                return [mk(c) for c in range(4)]

            # filler queues: fill[a] holds pair-a's projection matmuls
            # (populated as x chunks are emitted); fillers_wo holds W_o
            # tiles (populated as otn chunks complete in the a=3 phase).
            fill = {1: deque(), 2: deque(), 3: deque()}
            fillers_wo = deque()

            def pull(n):
                for _ in range(n):
                    if fillers_wo:
                        fillers_wo.popleft()()
                    elif fill[1]:
                        fill[1].popleft()()
                    elif fill[2]:
                        fill[2].popleft()()
                    elif fill[3]:
                        fill[3].popleft()()

            def drain(q):
                while q:
                    q.popleft()()

            # ---- attention block: head pair a, q-block j ---------------
            # Both heads' scores land in one 2-bank PSUM tile so a single
            # strided exp covers them. Depth-2 pipeline: AV(k0) is emitted
            # after S(k0+1), so the PE streams S(k0+1) plus fillers while
            # the scalar engine runs exp(k0).
            def attn_block(a, j, rate):
                av = {
                    hh: psum.tile([DK + 1, 512], f32, tag="ps",
                                  name=f"av{a}_{hh}_{j}")
                    for hh in (0, 1)
                }
                pend = None
                for k0 in range(4 * j + 4):
                    r = k0 - 4 * j
                    lo = 128 * r if r > 0 else 0
                    s_ps = psum2.tile([P, 2, 512], f32, tag="s2",
                                      name=f"sps{a}_{j}")
                    for hh in (0, 1):
                        poff = 64 * hh
                        nc.tensor.matmul(
                            s_ps[:, hh, lo:512],
                            lhsT=kt[a][poff:poff + 64, P * k0:P * k0 + P],
                            rhs=qt[a][poff:poff + 64,
                                      512 * j + lo:512 * j + 512],
                            start=True,
                            stop=True,
                        )
                    u_t = work.tile([P, 2, 512], bf16, tag="u", bufs=10,
                                    name=f"u{a}_{j}")
                    nc.scalar.activation(
                        u_t[:, :, lo:512], s_ps[:, :, lo:512], Exp,
                        scale=0.125,
                    )
                    if r >= 0:
                        # only the ragged 128-col sub-block needs masking
                        nc.vector.tensor_mul(
                            u_t[:, :, lo:lo + 128],
                            u_t[:, :, lo:lo + 128],
                            mask_sb,
                        )
                    # a filler between S(k0) and AV(k0-1) gives exp(k0-1)
                    # slack so the AV issue never stalls on the chain
                    pull(1)
                    if pend is not None:
                        pend()
                    pull(rate - 1)

                    def mk_av(k0=k0, lo=lo, u_t=u_t):
                        def op():
                            for hh in (0, 1):
                                nc.tensor.matmul(
                                    av[hh][:, lo:512],
                                    lhsT=vt[k0][:, 2 * a + hh, :],
                                    rhs=u_t[:, hh, lo:512],
                                    start=(k0 == 0),
                                    stop=(k0 == 4 * j + 3),
                                )
                        return op

                    pend = mk_av()
                pend()
                # ---- normalize: otn = av[:64] * bcast(1/Z) --------
                for hh in (0, 1):
                    poff = 64 * hh
                    z_sb = work.tile([1, 512], f32, tag="z", bufs=2,
                                     name=f"z{a}_{j}")
                    nc.vector.tensor_copy(z_sb, av[hh][DK:DK + 1, :])
                    rz = work.tile([1, 512], f32, tag="rz", bufs=2,
                                   name=f"rz{a}_{j}")
                    nc.vector.reciprocal_approx_fast(rz, z_sb)
                    bc = work.tile([64, 512], f32, tag="bc", bufs=2,
                                   name=f"bc{a}_{j}")
                    nc.gpsimd.partition_broadcast(bc, rz)
                    nc.vector.tensor_mul(
                        otn[a][poff:poff + 64, 512 * j:512 * j + 512],
                        av[hh][0:DK, :],
                        bc,
                    )

            # ---- chunk loop: upfront work + pair-0 attention -----------
            # Per chunk: V tiles, pair-0 projections, then attention block
            # (0, j=tch) — its S matmuls only need chunks <= tch. Later
            # pairs' projections for this chunk join the filler queues.
            for tch in range(NQ):
                for tt in range(4 * tch, 4 * tch + 4):
                    for op in v_tile_ops(tt):
                        op()
                for op in proj_tile_ops("qt", wq_sb, qt[0], 0, tch):
                    op()
                for op in proj_tile_ops("kt", wk_sb, kt[0], 0, tch):
                    op()
                for a in (1, 2, 3):
                    fill[a].extend(proj_tile_ops("qt", wq_sb, qt[a], a, tch))
                    fill[a].extend(proj_tile_ops("kt", wk_sb, kt[a], a, tch))
                attn_block(0, tch, rate=2)

            # ---- pairs 1-3 --------------------------------------------
            for a in (1, 2, 3):
                for aa in range(1, a + 1):
                    drain(fill[aa])
                for j in range(NQ):
                    attn_block(a, j, rate=2 if a < 3 else 3)
                    if a == 3 and j < 3:
                        for dt_ in range(8):
                            fillers_wo.extend(wo_tile_ops(dt_, j))

            # ---- output projection tail -------------------------------
            drain(fillers_wo)
            for dt_ in range(8):
                for op in wo_tile_ops(dt_, 3):
                    op()

    nc.finalize()
    return nc


def _get_nc():
    if "nc" not in _CACHE:
        _CACHE["nc"] = _build()
    return _CACHE["nc"]


def kernel(x, W_q, W_k, W_v, W_o):
    import ml_dtypes
    from concourse.bass_utils import run_bass_kernel_spmd

    bf16 = ml_dtypes.bfloat16
    x = np.asarray(x, dtype=np.float32)
    W_q = np.asarray(W_q, dtype=np.float32)
    W_k = np.asarray(W_k, dtype=np.float32)
    W_v = np.asarray(W_v, dtype=np.float32)
    W_o = np.asarray(W_o, dtype=np.float32)

    kk = np.arange(P)[:, None]
    cc = np.arange(P)[None, :]
    mask = np.tile((cc >= kk), (1, 2)).astype(bf16)

    in_maps = []
    for c in range(NCORES):
        b, g = c // 2, c % 2
        rows = slice(HD * g, HD * g + HD)
        in_maps.append(
            {
                "xT": np.ascontiguousarray(x[b].T).astype(bf16),
                "wqT": np.ascontiguousarray(W_q[rows, :].T).astype(bf16),
                "wkT": np.ascontiguousarray(W_k[rows, :].T).astype(bf16),
                "wvT": np.ascontiguousarray(W_v[rows, :].T).astype(bf16),
                "woT": np.ascontiguousarray(W_o[:, rows].T).astype(bf16),
                "mask": mask,
            }
        )

    res = run_bass_kernel_spmd(_get_nc(), in_maps, list(range(NCORES)))
    y = np.zeros((B, T, D), np.float32)
    for c in range(NCORES):
        y[c // 2] += res.results[c]["yT"].T.astype(np.float32)
    return y


# revision 20
# speedup vs baseline: 1.0099x; 1.0099x over previous
"""Multi-head causal attention on 8 TRN2 NeuronCores.

Sharding: core c -> (batch b = c//2, head-group g = c%2). Each core computes
Q/K/V projections for its 8 heads (512 of the 1024 channels), causal
attention, and the row-parallel W_o partial product; the host sums the two
partials per batch (the "all-reduce").

Device layouts (per core):
  xT   (1024, 2048) bf16   x[b] transposed (channels on partitions)
  wqT  (1024, 512)  bf16   W_q[rows g].T  -> lhsT for QT = Wq_g @ xT
  wkT  (1024, 512)  bf16   same for K
  wvT  (1024, 512)  bf16   rhs for natural-layout V = x @ Wv_g.T
  woT  (512, 1024)  bf16   W_o[:, cols g].T -> lhsT for yT = Wo_g @ O^T
  mask (128, 256)   bf16   the 128x128 causal triangle (q>=k), 2 head copies
  yT   (1024, 2048) bf16   partial output, transposed

Attention per head h (d_k=64): scores are computed transposed,
S^T = K_h @ Q_h^T (k on partitions, q on free axis), exp on the scalar
engine (no max subtraction: |scores/8| < ~6 at these scales), multiplicative
0/1 mask on the single ragged 128-col sub-block of diagonal tiles, and P^T
is consumed directly as the moving operand of out^T = [V_h | 1]^T @ P^T,
whose row 64 accumulates the softmax denominators Z. Diagonal-crossing
blocks are computed only on their valid column range. Heads run in pairs
(partition offsets 0/64) so the two K=64 score matmuls occupy disjoint PE
row-groups concurrently.

v2 pipeline: head-pair 0's attention blocks are interleaved into the input
chunk loop, so the scalar engine starts its ~190us of exp work ~15us into
the kernel instead of ~80us. The attention inner loop is depth-2 software
pipelined (S(k0+1) is emitted before AV(k0)) and pulls "filler" matmuls
(later pairs' Q^T/K^T projections, then W_o tiles) to keep the PE busy
through the S -> exp -> mask -> AV latency chain; a starved PE also gets
re-throttled to 1.2 GHz by the HAM clock gate, so density matters twice.
"""

from collections import deque

import numpy as np

B, T, D = 4, 2048, 1024
NH, DK = 16, 64
NCORES = 8
HPC = NH // 2            # heads per core
HD = HPC * DK            # 512 head-dim channels per core
P = 128                  # partitions
NT = T // P              # 16 k-tiles
NQ = T // 512            # 4 q-blocks

_CACHE = {}


def _build():
    import concourse.mybir as mybir
    import concourse.tile as tile
    from concourse import bacc
    from concourse.tile import add_dep_helper

    f32, bf16 = mybir.dt.float32, mybir.dt.bfloat16
    Exp = mybir.ActivationFunctionType.Exp

    nc = bacc.Bacc(None, target_bir_lowering=False, debug=False)
    xT = nc.dram_tensor("xT", [D, T], bf16, kind="ExternalInput")
    wqT = nc.dram_tensor("wqT", [D, HD], bf16, kind="ExternalInput")
    wkT = nc.dram_tensor("wkT", [D, HD], bf16, kind="ExternalInput")
    wvT = nc.dram_tensor("wvT", [D, HD], bf16, kind="ExternalInput")
    woT = nc.dram_tensor("woT", [HD, D], bf16, kind="ExternalInput")
    mask = nc.dram_tensor("mask", [P, 2 * P], bf16, kind="ExternalInput")
    yT = nc.dram_tensor("yT", [D, T], bf16, kind="ExternalOutput")

    with tile.TileContext(nc) as tc:
        with (
            tc.tile_pool(name="persist", bufs=1) as persist,
            tc.tile_pool(name="work", bufs=6) as work,
            tc.tile_pool(name="psum", bufs=4, space="PSUM") as psum,
            tc.tile_pool(name="psum2", bufs=2, space="PSUM") as psum2,
        ):
            # ---- persistent tiles --------------------------------------
            xall = [persist.tile([P, 8, 512], bf16, tag=f"xc{t}",
                                 name=f"xc{t}")
                    for t in range(NQ)]
            wq_sb = persist.tile([P, 8, HD], bf16, tag="wq")
            wk_sb = persist.tile([P, 8, HD], bf16, tag="wk")
            wv_sb = persist.tile([P, 8, HD], bf16, tag="wv")
            wo_sb = persist.tile([P, 4, D], bf16, tag="wo")
            mask_sb = persist.tile([P, 2, P], bf16, tag="mask")
            qt = [persist.tile([P, T], bf16, tag=f"qt{a}", name=f"qt{a}")
                  for a in range(4)]
            kt = [persist.tile([P, T], bf16, tag=f"kt{a}", name=f"kt{a}")
                  for a in range(4)]
            vt = [persist.tile([P, HPC, DK + 1], bf16, tag=f"v{tt}", name=f"v{tt}")
                  for tt in range(NT)]
            otn = [persist.tile([P, T], bf16, tag=f"otn{i}", name=f"otn{i}")
                   for i in range(4)]

            # ---- input DMAs -------------------------------------------
            # xT streams in t-chunk-major order (per-c chains) so the
            # upfront V / Q^T / K^T matmuls can start on early chunks
            # instead of waiting for the whole 4MB transfer. wq/wk/mask
            # follow chunk 0 (needed by the first attention block); wo
            # follows chunk 1 (not needed until the a=3 phase).
            nc.sync.dma_start(out=wv_sb, in_=wvT.rearrange("(co p) d -> p co d", p=P))
            xT_r = xT.rearrange("(co p) t -> p co t", p=P)
            xdma = {}

            def emit_x_chunk(tch):
                ins = nc.sync.dma_start(
                    out=xall[tch],
                    in_=xT_r[:, :, 512 * tch:512 * tch + 512],
                )
                if tch > 0:
                    add_dep_helper(ins.ins, xdma[tch - 1], sync=True,
                                   reason="x chunks in order")
                xdma[tch] = ins.ins

            def emit_w(dst, srcp, gate):
                if srcp is None:
                    ins = nc.sync.dma_start(
                        out=mask_sb,
                        in_=mask.rearrange("p (g q) -> p g q", g=2),
                    )
                else:
                    ins = nc.sync.dma_start(
                        out=dst, in_=srcp.rearrange("(co p) d -> p co d", p=P)
                    )
                add_dep_helper(ins.ins, xdma[gate], sync=True,
                               reason="weights after early xT chunks")

            emit_x_chunk(0)
            emit_w(wq_sb, wqT, 0)
            emit_w(wk_sb, wkT, 0)
            emit_w(mask_sb, None, 0)
            emit_x_chunk(1)
            emit_w(wo_sb, woT, 1)
            emit_x_chunk(2)
            emit_x_chunk(3)
            for tt in range(NT):
                nc.vector.memset(vt[tt][:, :, DK:DK + 1], 1.0)

            # ---- PE warmup: dummy matmuls on scratch data while the
            # input DMA streams, so the HAM clock gate un-throttles the
            # PE (1.2 -> 2.4 GHz takes ~3.4us of sustained activity)
            # before the first real matmul arrives.
            warm = persist.tile([P, 512], bf16, tag="warm")
            nc.vector.memset(warm, 0.0)
            for i in range(28):
                wps = psum.tile([P, 512], f32, tag="ps", name=f"warm{i}")
                nc.tensor.matmul(wps, lhsT=warm[:, 0:128], rhs=warm,
                                 start=True, stop=True)

            # ---- op builders (each closure emits one PE matmul) --------
            def v_tile_ops(tt):
                st = {}

                def mk(c):
                    def op():
                        if c == 0:
                            st["ps"] = psum.tile([P, HD], f32, tag="ps",
                                                 name=f"vps{tt}")
                        nc.tensor.matmul(
                            st["ps"],
                            lhsT=xall[tt // 4][:, c,
                                               P * (tt % 4):P * (tt % 4) + P],
                            rhs=wv_sb[:, c, :],
                            start=(c == 0),
                            stop=(c == 7),
                        )
                        if c == 7:
                            # scalar engine: idle during the chunk loop
                            nc.scalar.copy(
                                vt[tt][:, :, 0:DK],
                                st["ps"].rearrange("p (h e) -> p h e", e=DK),
                            )
                    return op

                return [mk(c) for c in range(8)]

            def proj_tile_ops(nm, w_sb, out_sb, a, tch, eng="vector"):
                st = {}

                def mk(c):
                    def op():
                        if c == 0:
                            st["ps"] = psum.tile([P, 512], f32, tag="ps",
                                                 name=f"{nm}ps{a}_{tch}")
                        nc.tensor.matmul(
                            st["ps"],
                            lhsT=w_sb[:, c, 128 * a:128 * a + 128],
                            rhs=xall[tch][:, c, :],
                            start=(c == 0),
                            stop=(c == 7),
                        )
                        if c == 7:
                            if eng == "scalar":
                                nc.scalar.copy(
                                    out_sb[:, 512 * tch:512 * tch + 512],
                                    st["ps"],
                                )
                            else:
                                nc.vector.tensor_copy(
                                    out_sb[:, 512 * tch:512 * tch + 512],
                                    st["ps"],
                                )
                    return op

                return [mk(c) for c in range(8)]

            def wo_tile_ops(dt_, tch, tail=False):
                st = {}

                def mk(c):
                    def op():
                        if c == 0:
                            # tail tiles use the attention-pipeline psum
                            # ring, idle once the last block's exps are
                            # done - the shared "ps" ring would make them
                            # wait on the live av accumulators.
                            if tail:
                                st["ps"] = psum2.tile([P, 512], f32,
                                                      tag="s2",
                                                      name=f"yps{dt_}_{tch}")
                            else:
                                st["ps"] = psum.tile([P, 512], f32, tag="ps",
                                                     name=f"yps{dt_}_{tch}")
                        nc.tensor.matmul(
                            st["ps"],
                            lhsT=wo_sb[:, c, 128 * dt_:128 * dt_ + 128],
                            rhs=otn[c][:, 512 * tch:512 * tch + 512],
                            start=(c == 0),
                            stop=(c == 3),
                        )
                        if c == 3:
                            yst = work.tile([P, 512], bf16, tag="yst", bufs=3,
                                            name=f"yst{dt_}_{tch}")
                            # scalar engine: idle in the W_o phase (exp done)
                            nc.scalar.copy(yst, st["ps"])
                            nc.sync.dma_start(
                                out=yT[128 * dt_:128 * dt_ + 128,
                                       512 * tch:512 * tch + 512],
                                in_=yst,
                            )
                    return op

                return [mk(c) for c in range(4)]

            # filler queues: fill[a] holds pair-a's projection matmuls
            # (populated as x chunks are emitted); fillers_wo holds W_o
            # tiles (populated as otn chunks complete in the a=3 phase).
            fill = {1: deque(), 2: deque(), 3: deque()}
            fillers_wo = deque()
            wo_reserve = [0]

            def pull(n):
                for _ in range(n):
                    if len(fillers_wo) > wo_reserve[0]:
                        fillers_wo.popleft()()
                    elif fill[1]:
                        fill[1].popleft()()
                    elif fill[2]:
                        fill[2].popleft()()
                    elif fill[3]:
                        fill[3].popleft()()

            def drain(q):
                while q:
                    q.popleft()()

            # ---- attention block: head pair a, q-block j ---------------
            # Both heads' scores land in one 2-bank PSUM tile so a single
            # strided exp covers them. Depth-2 pipeline: AV(k0) is emitted
            # after S(k0+1), so the PE streams S(k0+1) plus fillers while
            # the scalar engine runs exp(k0).
            def attn_block(a, j, rate, late_fill=None):
                av = {
                    hh: psum.tile([DK + 1, 512], f32, tag="ps",
                                  name=f"av{a}_{hh}_{j}")
                    for hh in (0, 1)
                }
                pend = None
                for k0 in range(4 * j + 4):
                    r = k0 - 4 * j
                    lo = 128 * r if r > 0 else 0
                    s_ps = psum2.tile([P, 2, 512], f32, tag="s2",
                                      name=f"sps{a}_{j}")
                    for hh in (0, 1):
                        poff = 64 * hh
                        nc.tensor.matmul(
                            s_ps[:, hh, lo:512],
                            lhsT=kt[a][poff:poff + 64, P * k0:P * k0 + P],
                            rhs=qt[a][poff:poff + 64,
                                      512 * j + lo:512 * j + 512],
                            start=True,
                            stop=True,
                        )
                    u_t = work.tile([P, 2, 512], bf16, tag="u", bufs=10,
                                    name=f"u{a}_{j}")
                    nc.scalar.activation(
                        u_t[:, :, lo:512], s_ps[:, :, lo:512], Exp,
                        scale=0.125,
                    )
                    if r >= 0:
                        # only the ragged 128-col sub-block needs masking
                        nc.vector.tensor_mul(
                            u_t[:, :, lo:lo + 128],
                            u_t[:, :, lo:lo + 128],
                            mask_sb,
                        )
                    # a filler between S(k0) and AV(k0-1) gives exp(k0-1)
                    # slack so the AV issue never stalls on the chain
                    pull(1)
                    if pend is not None:
                        pend()
                    pull(rate - 1)

                    def mk_av(k0=k0, lo=lo, u_t=u_t):
                        def op():
                            for hh in (0, 1):
                                nc.tensor.matmul(
                                    av[hh][:, lo:512],
                                    lhsT=vt[k0][:, 2 * a + hh, :],
                                    rhs=u_t[:, hh, lo:512],
                                    start=(k0 == 0),
                                    stop=(k0 == 4 * j + 3),
                                )
                        return op

                    pend = mk_av()
                pend()
                # ---- normalize: otn = av[:64] * bcast(1/Z) --------
                for hh in (0, 1):
                    poff = 64 * hh
                    z_sb = work.tile([1, 512], f32, tag="z", bufs=2,
                                     name=f"z{a}_{j}")
                    nc.vector.tensor_copy(z_sb, av[hh][DK:DK + 1, :])
                    rz = work.tile([1, 512], f32, tag="rz", bufs=2,
                                   name=f"rz{a}_{j}")
                    nc.vector.reciprocal_approx_fast(rz, z_sb)
                    bc = work.tile([64, 512], f32, tag="bc", bufs=2,
                                   name=f"bc{a}_{j}")
                    nc.gpsimd.partition_broadcast(bc, rz)
                    nc.vector.tensor_mul(
                        otn[a][poff:poff + 64, 512 * j:512 * j + 512],
                        av[hh][0:DK, :],
                        bc,
                    )

            # ---- chunk loop: upfront work + pair-0 attention -----------
            # Per chunk: V tiles, pair-0 projections, then attention block
            # (0, j=tch) — its S matmuls only need chunks <= tch. Later
            # pairs' projections for this chunk join the filler queues.
            for tch in range(NQ):
                for tt in range(4 * tch, 4 * tch + 4):
                    for op in v_tile_ops(tt):
                        op()
                for op in proj_tile_ops("qt", wq_sb, qt[0], 0, tch,
                                        eng="scalar"):
                    op()
                for op in proj_tile_ops("kt", wk_sb, kt[0], 0, tch,
                                        eng="scalar"):
                    op()
                for a in (1, 2, 3):
                    fill[a].extend(proj_tile_ops("qt", wq_sb, qt[a], a, tch))
                    fill[a].extend(proj_tile_ops("kt", wk_sb, kt[a], a, tch))
                attn_block(0, tch, rate=2)

            # ---- pairs 1-3 --------------------------------------------
            for a in (1, 2, 3):
                for aa in range(1, a + 1):
                    drain(fill[aa])
                for j in range(NQ):
                    if a == 3 and j == 3:
                        # hold back W_o tiles to bridge the PE across the
                        # final normalize chain (else it goes HAM-cold and
                        # the whole tail runs at half clock)
                        wo_reserve[0] = 12
                    attn_block(a, j, rate=2 if a < 3 else 3)
                    if a == 3 and j < 3:
                        for dt_ in range(8):
                            fillers_wo.extend(
                                wo_tile_ops(dt_, j, tail=(j == 2 and dt_ >= 5))
                            )

            # ---- output projection tail -------------------------------
            wo_reserve[0] = 0
            drain(fillers_wo)
            for dt_ in range(8):
                for op in wo_tile_ops(dt_, 3, tail=True):
                    op()

    nc.finalize()
    return nc


def _get_nc():
    if "nc" not in _CACHE:
        _CACHE["nc"] = _build()
    return _CACHE["nc"]


def kernel(x, W_q, W_k, W_v, W_o):
    import ml_dtypes
    from concourse.bass_utils import run_bass_kernel_spmd

    bf16 = ml_dtypes.bfloat16
    x = np.asarray(x, dtype=np.float32)
    W_q = np.asarray(W_q, dtype=np.float32)
    W_k = np.asarray(W_k, dtype=np.float32)
    W_v = np.asarray(W_v, dtype=np.float32)
    W_o = np.asarray(W_o, dtype=np.float32)

    kk = np.arange(P)[:, None]
    cc = np.arange(P)[None, :]
    mask = np.tile((cc >= kk), (1, 2)).astype(bf16)

    in_maps = []
    for c in range(NCORES):
        b, g = c // 2, c % 2
        rows = slice(HD * g, HD * g + HD)
        in_maps.append(
            {
                "xT": np.ascontiguousarray(x[b].T).astype(bf16),
                "wqT": np.ascontiguousarray(W_q[rows, :].T).astype(bf16),
                "wkT": np.ascontiguousarray(W_k[rows, :].T).astype(bf16),
                "wvT": np.ascontiguousarray(W_v[rows, :].T).astype(bf16),
                "woT": np.ascontiguousarray(W_o[:, rows].T).astype(bf16),
                "mask": mask,
            }
        )

    res = run_bass_kernel_spmd(_get_nc(), in_maps, list(range(NCORES)))
    y = np.zeros((B, T, D), np.float32)
    for c in range(NCORES):
        y[c // 2] += res.results[c]["yT"].T.astype(np.float32)
    return y


# revision 21
# speedup vs baseline: 1.2137x; 1.2018x over previous
"""Multi-head causal attention on 8 TRN2 NeuronCores.

Sharding: core c -> (batch b = c//2, head-group g = c%2). Each core computes
Q/K/V projections for its 8 heads (512 of the 1024 channels), causal
attention, and the row-parallel W_o partial product; the host sums the two
partials per batch (the "all-reduce").

Device layouts (per core):
  xT   (1024, 2048) bf16   x[b] transposed (channels on partitions)
  wqT  (1024, 512)  bf16   W_q[rows g].T  -> lhsT for QT = Wq_g @ xT
  wkT  (1024, 512)  bf16   same for K
  wvT  (1024, 512)  bf16   rhs for natural-layout V = x @ Wv_g.T
  woT  (512, 1024)  bf16   W_o[:, cols g].T -> lhsT for yT = Wo_g @ O^T
  mask (128, 256)   bf16   the 128x128 causal triangle (q>=k), 2 head copies
  yT   (1024, 2048) bf16   partial output, transposed

Attention per head h (d_k=64): scores are computed transposed,
S^T = K_h @ Q_h^T (k on partitions, q on free axis), exp on the scalar
engine (no max subtraction: |scores/8| < ~6 at these scales), multiplicative
0/1 mask on the single ragged 128-col sub-block of diagonal tiles, and P^T
is consumed directly as the moving operand of out^T = [V_h | 1]^T @ P^T,
whose row 64 accumulates the softmax denominators Z. Diagonal-crossing
blocks are computed only on their valid column range. Heads run in pairs
(partition offsets 0/64) so the two K=64 score matmuls occupy disjoint PE
row-groups concurrently.

Pipeline (single pass, ~283us measured; ~192us pure matmul-stream floor):
  - head-pair 0's attention blocks are interleaved into the input chunk
    loop, so the scalar engine starts its ~160us of exp work ~15us into
    the kernel instead of ~80us;
  - the attention inner loop is depth-2 software pipelined (S(k0+1) is
    emitted before AV(k0), with a filler matmul between them) and pulls
    "filler" matmuls (later pairs' Q^T/K^T projections, then W_o tiles)
    so the PE never waits on the S -> exp -> mask -> AV chain; a starved
    PE is also re-throttled to 1.2 GHz by the HAM clock gate (~3.4us idle
    window), so density matters twice;
  - dummy warmup matmuls run during the DMA ramp so the HAM gate opens
    before real work arrives; weights DMA between x chunks 0 and 1;
  - pair-0 projection / V evacuations run on the (then idle) scalar
    engine; W_o evacuations likewise (scalar is idle once exps end);
  - 12 W_o tiles are held in reserve to bridge the PE across the final
    softmax-normalize chain, and the tail W_o PSUM tiles reuse the
    attention score ring (idle by then) instead of colliding with the
    live AV accumulators.
"""

from collections import deque

import numpy as np

B, T, D = 4, 2048, 1024
NH, DK = 16, 64
NCORES = 8
HPC = NH // 2            # heads per core
HD = HPC * DK            # 512 head-dim channels per core
P = 128                  # partitions
NT = T // P              # 16 k-tiles
NQ = T // 512            # 4 q-blocks

_CACHE = {}


def _build():
    import concourse.mybir as mybir
    import concourse.tile as tile
    from concourse import bacc
    from concourse.tile import add_dep_helper

    f32, bf16 = mybir.dt.float32, mybir.dt.bfloat16
    Exp = mybir.ActivationFunctionType.Exp

    nc = bacc.Bacc(None, target_bir_lowering=False, debug=False)
    xT = nc.dram_tensor("xT", [D, T], bf16, kind="ExternalInput")
    wqT = nc.dram_tensor("wqT", [D, HD], bf16, kind="ExternalInput")
    wkT = nc.dram_tensor("wkT", [D, HD], bf16, kind="ExternalInput")
    wvT = nc.dram_tensor("wvT", [D, HD], bf16, kind="ExternalInput")
    woT = nc.dram_tensor("woT", [HD, D], bf16, kind="ExternalInput")
    mask = nc.dram_tensor("mask", [P, 2 * P], bf16, kind="ExternalInput")
    yT = nc.dram_tensor("yT", [D, T], bf16, kind="ExternalOutput")

    with tile.TileContext(nc) as tc:
        with (
            tc.tile_pool(name="persist", bufs=1) as persist,
            tc.tile_pool(name="work", bufs=6) as work,
            tc.tile_pool(name="psum", bufs=4, space="PSUM") as psum,
            tc.tile_pool(name="psum2", bufs=2, space="PSUM") as psum2,
        ):
            # ---- persistent tiles --------------------------------------
            xall = [persist.tile([P, 8, 512], bf16, tag=f"xc{t}",
                                 name=f"xc{t}")
                    for t in range(NQ)]
            wq_sb = persist.tile([P, 8, HD], bf16, tag="wq")
            wk_sb = persist.tile([P, 8, HD], bf16, tag="wk")
            wv_sb = persist.tile([P, 8, HD], bf16, tag="wv")
            wo_sb = persist.tile([P, 4, D], bf16, tag="wo")
            mask_sb = persist.tile([P, 2, P], bf16, tag="mask")
            qt = [persist.tile([P, T], bf16, tag=f"qt{a}", name=f"qt{a}")
                  for a in range(4)]
            kt = [persist.tile([P, T], bf16, tag=f"kt{a}", name=f"kt{a}")
                  for a in range(4)]
            vt = [persist.tile([P, HPC, DK + 1], bf16, tag=f"v{tt}", name=f"v{tt}")
                  for tt in range(NT)]
            otn = [persist.tile([P, T], bf16, tag=f"otn{i}", name=f"otn{i}")
                   for i in range(4)]

            # ---- input DMAs -------------------------------------------
            # xT streams in t-chunk-major order (per-c chains) so the
            # upfront V / Q^T / K^T matmuls can start on early chunks
            # instead of waiting for the whole 4MB transfer. wq/wk/mask
            # follow chunk 0 (needed by the first attention block); wo
            # follows chunk 1 (not needed until the a=3 phase).
            nc.sync.dma_start(out=wv_sb, in_=wvT.rearrange("(co p) d -> p co d", p=P))
            xT_r = xT.rearrange("(co p) t -> p co t", p=P)
            xdma = {}

            def emit_x_chunk(tch):
                ins = nc.sync.dma_start(
                    out=xall[tch],
                    in_=xT_r[:, :, 512 * tch:512 * tch + 512],
                )
                if tch > 0:
                    add_dep_helper(ins.ins, xdma[tch - 1], sync=True,
                                   reason="x chunks in order")
                xdma[tch] = ins.ins

            def emit_w(dst, srcp, gate):
                if srcp is None:
                    ins = nc.sync.dma_start(
                        out=mask_sb,
                        in_=mask.rearrange("p (g q) -> p g q", g=2),
                    )
                else:
                    ins = nc.sync.dma_start(
                        out=dst, in_=srcp.rearrange("(co p) d -> p co d", p=P)
                    )
                add_dep_helper(ins.ins, xdma[gate], sync=True,
                               reason="weights after early xT chunks")

            emit_x_chunk(0)
            emit_w(wq_sb, wqT, 0)
            emit_w(wk_sb, wkT, 0)
            emit_w(mask_sb, None, 0)
            emit_x_chunk(1)
            emit_w(wo_sb, woT, 1)
            emit_x_chunk(2)
            emit_x_chunk(3)
            for tt in range(NT):
                nc.vector.memset(vt[tt][:, :, DK:DK + 1], 1.0)

            # ---- PE warmup: dummy matmuls on scratch data while the
            # input DMA streams, so the HAM clock gate un-throttles the
            # PE (1.2 -> 2.4 GHz takes ~3.4us of sustained activity)
            # before the first real matmul arrives.
            warm = persist.tile([P, 512], bf16, tag="warm")
            nc.vector.memset(warm, 0.0)
            for i in range(28):
                wps = psum.tile([P, 512], f32, tag="ps", name=f"warm{i}")
                nc.tensor.matmul(wps, lhsT=warm[:, 0:128], rhs=warm,
                                 start=True, stop=True)

            # ---- op builders (each closure emits one PE matmul) --------
            def v_tile_ops(tt):
                st = {}

                def mk(c):
                    def op():
                        if c == 0:
                            st["ps"] = psum.tile([P, HD], f32, tag="ps",
                                                 name=f"vps{tt}")
                        nc.tensor.matmul(
                            st["ps"],
                            lhsT=xall[tt // 4][:, c,
                                               P * (tt % 4):P * (tt % 4) + P],
                            rhs=wv_sb[:, c, :],
                            start=(c == 0),
                            stop=(c == 7),
                        )
                        if c == 7:
                            # scalar engine: idle during the chunk loop
                            nc.scalar.copy(
                                vt[tt][:, :, 0:DK],
                                st["ps"].rearrange("p (h e) -> p h e", e=DK),
                            )
                    return op

                return [mk(c) for c in range(8)]

            def proj_tile_ops(nm, w_sb, out_sb, a, tch, eng="vector"):
                st = {}

                def mk(c):
                    def op():
                        if c == 0:
                            st["ps"] = psum.tile([P, 512], f32, tag="ps",
                                                 name=f"{nm}ps{a}_{tch}")
                        nc.tensor.matmul(
                            st["ps"],
                            lhsT=w_sb[:, c, 128 * a:128 * a + 128],
                            rhs=xall[tch][:, c, :],
                            start=(c == 0),
                            stop=(c == 7),
                        )
                        if c == 7:
                            if eng == "scalar":
                                nc.scalar.copy(
                                    out_sb[:, 512 * tch:512 * tch + 512],
                                    st["ps"],
                                )
                            else:
                                nc.vector.tensor_copy(
                                    out_sb[:, 512 * tch:512 * tch + 512],
                                    st["ps"],
                                )
                    return op

                return [mk(c) for c in range(8)]

            def wo_tile_ops(dt_, tch, tail=False):
                st = {}

                def mk(c):
                    def op():
                        if c == 0:
                            # tail tiles use the attention-pipeline psum
                            # ring, idle once the last block's exps are
                            # done - the shared "ps" ring would make them
                            # wait on the live av accumulators.
                            if tail:
                                st["ps"] = psum2.tile([P, 512], f32,
                                                      tag="s2",
                                                      name=f"yps{dt_}_{tch}")
                            else:
                                st["ps"] = psum.tile([P, 512], f32, tag="ps",
                                                     name=f"yps{dt_}_{tch}")
                        nc.tensor.matmul(
                            st["ps"],
                            lhsT=wo_sb[:, c, 128 * dt_:128 * dt_ + 128],
                            rhs=otn[c][:, 512 * tch:512 * tch + 512],
                            start=(c == 0),
                            stop=(c == 3),
                        )
                        if c == 3:
                            yst = work.tile([P, 512], bf16, tag="yst", bufs=3,
                                            name=f"yst{dt_}_{tch}")
                            # scalar engine: idle in the W_o phase (exp done)
                            nc.scalar.copy(yst, st["ps"])
                            nc.sync.dma_start(
                                out=yT[128 * dt_:128 * dt_ + 128,
                                       512 * tch:512 * tch + 512],
                                in_=yst,
                            )
                    return op

                return [mk(c) for c in range(4)]

            # filler queues: fill[a] holds pair-a's projection matmuls
            # (populated as x chunks are emitted); fillers_wo holds W_o
            # tiles (populated as otn chunks complete in the a=3 phase).
            fill = {1: deque(), 2: deque(), 3: deque()}
            fillers_wo = deque()
            wo_reserve = [0]

            def pull(n):
                for _ in range(n):
                    if len(fillers_wo) > wo_reserve[0]:
                        fillers_wo.popleft()()
                    elif fill[1]:
                        fill[1].popleft()()
                    elif fill[2]:
                        fill[2].popleft()()
                    elif fill[3]:
                        fill[3].popleft()()

            def drain(q):
                while q:
                    q.popleft()()

            # ---- attention block: head pair a, q-block j ---------------
            # Both heads' scores land in one 2-bank PSUM tile so a single
            # strided exp covers them. Depth-2 pipeline: AV(k0) is emitted
            # after S(k0+1), so the PE streams S(k0+1) plus fillers while
            # the scalar engine runs exp(k0).
            def attn_block(a, j, rate, late_fill=None):
                av = {
                    hh: psum.tile([DK + 1, 512], f32, tag="ps",
                                  name=f"av{a}_{hh}_{j}")
                    for hh in (0, 1)
                }
                pend = None
                for k0 in range(4 * j + 4):
                    r = k0 - 4 * j
                    lo = 128 * r if r > 0 else 0
                    s_ps = psum2.tile([P, 2, 512], f32, tag="s2",
                                      name=f"sps{a}_{j}")
                    for hh in (0, 1):
                        poff = 64 * hh
                        nc.tensor.matmul(
                            s_ps[:, hh, lo:512],
                            lhsT=kt[a][poff:poff + 64, P * k0:P * k0 + P],
                            rhs=qt[a][poff:poff + 64,
                                      512 * j + lo:512 * j + 512],
                            start=True,
                            stop=True,
                        )
                    u_t = work.tile([P, 2, 512], bf16, tag="u", bufs=10,
                                    name=f"u{a}_{j}")
                    nc.scalar.activation(
                        u_t[:, :, lo:512], s_ps[:, :, lo:512], Exp,
                        scale=0.125,
                    )
                    if r >= 0:
                        # only the ragged 128-col sub-block needs masking
                        nc.vector.tensor_mul(
                            u_t[:, :, lo:lo + 128],
                            u_t[:, :, lo:lo + 128],
                            mask_sb,
                        )
                    # a filler between S(k0) and AV(k0-1) gives exp(k0-1)
                    # slack so the AV issue never stalls on the chain
                    pull(1)
                    if pend is not None:
                        pend()
                    pull(rate - 1)

                    def mk_av(k0=k0, lo=lo, u_t=u_t):
                        def op():
                            for hh in (0, 1):
                                nc.tensor.matmul(
                                    av[hh][:, lo:512],
                                    lhsT=vt[k0][:, 2 * a + hh, :],
                                    rhs=u_t[:, hh, lo:512],
                                    start=(k0 == 0),
                                    stop=(k0 == 4 * j + 3),
                                )
                        return op

                    pend = mk_av()
                pend()
                # ---- normalize: otn = av[:64] * bcast(1/Z) --------
                for hh in (0, 1):
                    poff = 64 * hh
                    z_sb = work.tile([1, 512], f32, tag="z", bufs=2,
                                     name=f"z{a}_{j}")
                    nc.vector.tensor_copy(z_sb, av[hh][DK:DK + 1, :])
                    rz = work.tile([1, 512], f32, tag="rz", bufs=2,
                                   name=f"rz{a}_{j}")
                    nc.vector.reciprocal_approx_fast(rz, z_sb)
                    bc = work.tile([64, 512], f32, tag="bc", bufs=2,
                                   name=f"bc{a}_{j}")
                    nc.gpsimd.partition_broadcast(bc, rz)
                    nc.vector.tensor_mul(
                        otn[a][poff:poff + 64, 512 * j:512 * j + 512],
                        av[hh][0:DK, :],
                        bc,
                    )

            # ---- chunk loop: upfront work + pair-0 attention -----------
            # Per chunk: V tiles, pair-0 projections, then attention block
            # (0, j=tch) — its S matmuls only need chunks <= tch. Later
            # pairs' projections for this chunk join the filler queues.
            for tch in range(NQ):
                for tt in range(4 * tch, 4 * tch + 4):
                    for op in v_tile_ops(tt):
                        op()
                for op in proj_tile_ops("qt", wq_sb, qt[0], 0, tch,
                                        eng="scalar"):
                    op()
                for op in proj_tile_ops("kt", wk_sb, kt[0], 0, tch,
                                        eng="scalar"):
                    op()
                for a in (1, 2, 3):
                    fill[a].extend(proj_tile_ops("qt", wq_sb, qt[a], a, tch))
                    fill[a].extend(proj_tile_ops("kt", wk_sb, kt[a], a, tch))
                attn_block(0, tch, rate=2)

            # ---- pairs 1-3 --------------------------------------------
            for a in (1, 2, 3):
                for aa in range(1, a + 1):
                    drain(fill[aa])
                for j in range(NQ):
                    if a == 3 and j == 3:
                        # hold back W_o tiles to bridge the PE across the
                        # final normalize chain (else it goes HAM-cold and
                        # the whole tail runs at half clock)
                        wo_reserve[0] = 12
                    attn_block(a, j, rate=2 if a < 3 else 3)
                    if a == 3 and j < 3:
                        for dt_ in range(8):
                            fillers_wo.extend(
                                wo_tile_ops(dt_, j, tail=(j == 2 and dt_ >= 5))
                            )

            # ---- output projection tail -------------------------------
            wo_reserve[0] = 0
            drain(fillers_wo)
            for dt_ in range(8):
                for op in wo_tile_ops(dt_, 3, tail=True):
                    op()

    nc.finalize()
    return nc


def _get_nc():
    if "nc" not in _CACHE:
        _CACHE["nc"] = _build()
    return _CACHE["nc"]


def kernel(x, W_q, W_k, W_v, W_o):
    import ml_dtypes
    from concourse.bass_utils import run_bass_kernel_spmd

    bf16 = ml_dtypes.bfloat16
    x = np.asarray(x, dtype=np.float32)
    W_q = np.asarray(W_q, dtype=np.float32)
    W_k = np.asarray(W_k, dtype=np.float32)
    W_v = np.asarray(W_v, dtype=np.float32)
    W_o = np.asarray(W_o, dtype=np.float32)

    kk = np.arange(P)[:, None]
    cc = np.arange(P)[None, :]
    mask = np.tile((cc >= kk), (1, 2)).astype(bf16)

    in_maps = []
    for c in range(NCORES):
        b, g = c // 2, c % 2
        rows = slice(HD * g, HD * g + HD)
        in_maps.append(
            {
                "xT": np.ascontiguousarray(x[b].T).astype(bf16),
                "wqT": np.ascontiguousarray(W_q[rows, :].T).astype(bf16),
                "wkT": np.ascontiguousarray(W_k[rows, :].T).astype(bf16),
                "wvT": np.ascontiguousarray(W_v[rows, :].T).astype(bf16),
                "woT": np.ascontiguousarray(W_o[:, rows].T).astype(bf16),
                "mask": mask,
            }
        )

    res = run_bass_kernel_spmd(_get_nc(), in_maps, list(range(NCORES)))
    y = np.zeros((B, T, D), np.float32)
    for c in range(NCORES):
        y[c // 2] += res.results[c]["yT"].T.astype(np.float32)
    return y
